# revision 1
# baseline (speedup 1.0000x reference)
"""CirLinear Trainium2 kernel: y = x @ build_weight(W, alphas, gumbels)^T + bias.

Strategy (8 NeuronCores, no collectives), 2x4 grid:
 - core c = tshard*4 + oshard: tokens [8192*tshard, +8192), out rows
   [512*oshard, +512)
 - weight build via hierarchical straight-diagonal pyramid:
   * up-sweep: straight diag partial sums s_b/t_b compose 2x per level
     (s_2b = s_b(00)+s_b(11)+t_b(01) etc), so cyclic diag sums for all 6
     scales cost ~2 passes over the data instead of one pass per scale
   * down-sweep: per-scale contributions pushed down a signed-lag pyramid
     G_b[U,V,lam], lam = s-r in (-b,b); parent lag = lam + b*(Vpar-Upar)
     needs no wrap handling
   * layout: partition = (P in-col-pair 16, q out-block 8); free =
     (r, phat, s); all scales operate free-dim only
 - acc (bf16 W_eff chunk) -> flat DRAM store (contiguous per partition,
   4 pipelined groups) -> 16 HW DMA-transposes [512,128] -> wT[ic] matmul
   lhsT tiles; no descriptor-heavy scatter, no big serial DVE phase
 - bf16 matmul (lhsT = wT[ic][:,128o:+128], rhs = xT tile [128i,512t])
   with fp32 PSUM accumulation over 16 K-chunks, bias added on the
   scalar engine, bf16 output out^T [512, 8192], host casts to f32
"""
import sys

sys.path.insert(0, '/opt/trn_rl_repo')

import numpy as np

import concourse.bass as bass
from concourse import bacc
import concourse.mybir as mybir
from concourse.tile import TileContext
from concourse.bass_utils import run_bass_kernel_spmd

N_CORES = 8
T_SHARDS, O_SHARDS = 2, 4
BATCH, TOKENS, IN_F, OUT_F = 16, 1024, 2048, 2048
TOK_TOTAL = BATCH * TOKENS            # 16384
TOK = TOK_TOTAL // T_SHARDS           # 8192 tokens per core
ROWS = OUT_F // O_SHARDS              # 512 out-features per core
N_IC = IN_F // 128                    # 16 contraction chunks
N_TG = TOK // 512                     # 16 token groups of 512
N_OS = ROWS // 128                    # 4 output-row subtiles
SCALES = [2, 4, 8, 16, 32, 64]        # alphas idx 1..6; idx 0 = identity

bf16 = mybir.dt.bfloat16
f32 = mybir.dt.float32
ADD = mybir.AluOpType.add
MULT = mybir.AluOpType.mult

_CACHE = {}


def _ap(t, part0, nparts, free_off, dims):
    """SBUF AP: partitions [part0, part0+nparts), free dims (stride,count)."""
    h = t.ap()
    fs = h.ap[0][0]  # per-partition free span
    return bass.AP(tensor=h.tensor, offset=part0 * fs + free_off,
                   ap=[[fs, nparts]] + [list(d) for d in dims])


def _build_nc(tok=TOK, debug_wflat=False, reps=1):
    n_tg = tok // 512
    nc = bacc.Bacc("TRN2", target_bir_lowering=False, debug=False, num_devices=N_CORES)
    # x pre-tiled on host as [tg, partition, ic, t]: every per-tg load is one
    # contiguous 16KB run per partition (128 descriptors, not 2048)
    xT = nc.dram_tensor("xT", [n_tg * 128, N_IC * 512], bf16, kind="ExternalInput")
    wsb = nc.dram_tensor("wsb", [128, 8192], bf16, kind="ExternalInput")
    bias_s = nc.dram_tensor("bias_s", [1, ROWS], f32, kind="ExternalInput")
    alphas = nc.dram_tensor("alphas", [1, 7], f32, kind="ExternalInput")
    gumbels = nc.dram_tensor("gumbels", [1, 7], f32, kind="ExternalInput")
    out = nc.dram_tensor("out", [ROWS, tok], bf16, kind="ExternalOutput")
    if debug_wflat:
        w_flat = nc.dram_tensor("w_flat", [128, 8192], bf16, kind="ExternalOutput")
    else:
        w_flat = nc.dram_tensor("w_flat", [128, 8192], bf16)

    with TileContext(nc) as tc:
        # ---------- load W chunk in build layout (first: gates the build) ----------
        # partition pi = P*8 + q (P in-col pair, q out 64-block); free (r, phat, s)
        # sync HWDGE queue; xt prefetches ride the scalar HWDGE queue so they
        # cannot delay this load
        wb_t = nc.alloc_sbuf_tensor("wb", [128, 8192], bf16)
        with tc.high_priority():
            # two free-half loads so the first half of the up-sweep can start
            # as soon as the first MB lands
            wsb_insts = [
                nc.sync.dma_start(
                    out=_ap(wb_t, 0, 128, hf * 4096, [[1, 4096]]),
                    in_=bass.AP(tensor=wsb, offset=hf * 4096,
                                ap=[[8192, 128], [1, 4096]]))
                for hf in range(2)]

        # ---------- softmax(alphas + gumbels) broadcast to 128 partitions ----------
        asb = nc.alloc_sbuf_tensor("asb", [128, 7], f32).ap()
        gsb = nc.alloc_sbuf_tensor("gsb", [128, 7], f32).ap()
        a_bc = nc.alloc_sbuf_tensor("a_bc", [128, 7], f32).ap()
        ssum = nc.alloc_sbuf_tensor("ssum", [128, 1], f32).ap()
        nc.gpsimd.dma_start(out=asb, in_=bass.AP(tensor=alphas, offset=0, ap=[[0, 128], [1, 7]]))
        nc.gpsimd.dma_start(out=gsb, in_=bass.AP(tensor=gumbels, offset=0, ap=[[0, 128], [1, 7]]))
        nc.vector.tensor_tensor(out=asb, in0=asb, in1=gsb, op=ADD)
        nc.scalar.activation(out=asb, in_=asb, func=mybir.ActivationFunctionType.Exp)
        nc.vector.tensor_reduce(out=ssum, in_=asb, axis=mybir.AxisListType.X, op=ADD)
        nc.vector.reciprocal(out=ssum, in_=ssum)
        nc.vector.tensor_scalar_mul(a_bc, asb, ssum)

        # ---------- bias: [1, 512] -> [128 part, 4] (per-osub per-partition) ----------
        bias_sb = nc.alloc_sbuf_tensor("bias_sb", [128, N_OS], f32).ap()
        with nc.allow_non_contiguous_dma(reason="512-element one-time bias transpose"):
            nc.gpsimd.dma_start(out=bias_sb, in_=bass.AP(tensor=bias_s, offset=0, ap=[[1, 128], [128, N_OS]]))

        # ---------- pyramid buffers ----------
        # S/T scale b: free off(u_ext, v, k) = u_ext*64 + v*b + k, size 8192/b
        S, T = {}, {}
        for b in SCALES:
            S[b] = nc.alloc_sbuf_tensor(f"S{b}", [128, 8192 // b], bf16)
            T[b] = nc.alloc_sbuf_tensor(f"T{b}", [128, 8192 // b], bf16)
        # Ghat scale b: off(u_ext, v, khat) = u_ext*128 + v*2b + khat, size 16384/b
        # (scale 2 is replaced by the E0/E1 interleaved arrays)
        G = {}
        for b in SCALES[1:]:
            G[b] = nc.alloc_sbuf_tensor(f"G{b}", [128, 16384 // b], bf16)
        tmp_t = nc.alloc_sbuf_tensor("tmpu", [128, 2048], bf16)
        d_t = nc.alloc_sbuf_tensor("dbuf", [128, 4096], bf16)
        t2_t = nc.alloc_sbuf_tensor("t2buf", [128, 4096], bf16)
        acc_t = nc.alloc_sbuf_tensor("acc", [128, 8192], bf16)
        # s-interleaved bottom-level arrays: E{er}[(2*r2+phat)*64 + s] holds
        # G2 value at lag (s%2 - er); lets the final adds run stride-1 at 2x
        E0_t = nc.alloc_sbuf_tensor("E0", [128, 4096], bf16)
        E1_t = nc.alloc_sbuf_tensor("E1", [128, 4096], bf16)

        def rd(t, b, du, dv, k0, kn):
            # read scale-b S/T array over parent raster (U, phat+V merged, k)
            return _ap(t.tensor if isinstance(t, bass.AP) else t, 0, 128,
                       128 * du + b * dv + k0,
                       [[256, 32 // b], [2 * b, 64 // b], [1, kn]])

        def wr(t, B, h):
            # write scale-B S/T half h (k in [h*b, h*b+b)), b = B//2
            b = B // 2
            return _ap(t, 0, 128, h * b, [[128, 32 // b], [2 * b, 64 // b], [1, b]])

        def tmpap(b):
            return _ap(tmp_t, 0, 128, 0, [[64, 32 // b], [b, 64 // b], [1, b]])

        # per-scale t2 = (S_b+T_b)*(a_b/b) arrays, packed into d_t/t2_t scratch
        _t2base = {2: (t2_t, 0), 4: (d_t, 0), 8: (d_t, 2048), 16: (d_t, 3072),
                   32: (d_t, 3584), 64: (d_t, 3840)}

        def t2full(b):
            buf, base = _t2base[b]
            return _ap(buf, 0, 128, base, [[1, 8192 // b]])

        def t2rd(b, off, dims):
            buf, base = _t2base[b]
            return _ap(buf, 0, 128, base + off, dims)

        def emit_t2(Bs):
            # Pool computes S+T (it is idle during the build); DVE scales in
            # place at 4x. Both are off the DVE up-sweep critical path.
            idx = SCALES.index(Bs) + 1
            nc.gpsimd.tensor_tensor(out=t2full(Bs), in0=_ap(S[Bs], 0, 128, 0, [[1, 8192 // Bs]]),
                                    in1=_ap(T[Bs], 0, 128, 0, [[1, 8192 // Bs]]), op=ADD)
            nc.vector.tensor_scalar(out=t2full(Bs), in0=t2full(Bs),
                                    scalar1=a_bc[:, idx:idx + 1], scalar2=1.0 / Bs,
                                    op0=MULT, op1=MULT)

        TTv = nc.vector.tensor_tensor
        CPs = nc.scalar.copy
        CPv = nc.vector.tensor_copy

        wT = [nc.alloc_sbuf_tensor(f"wT{ic}", [128, ROWS], bf16).ap() for ic in range(N_IC)]

        def dfull(t, b):
            return _ap(t, 0, 128, 0, [[1, 8192 // b]])

        def _one_pass(xt_pool, psum_pool, osb_pool):
            # ---------- up-sweep ----------
            # high_priority: the build chain must lead every engine's stream,
            # ahead of softmax and the xt prefetch issues
            with tc.high_priority():
                # level 1 -> 2 (children are wb; T_1 = 0), split in U-halves so
                # each half only waits on its own wsb half-load
                nc.gpsimd.memset(T[2].ap(), 0.0)
                for hf in range(2):
                    ro, wo = hf * 4096, hf * 2048
                    dr = [[256, 16], [2, 64], [1, 1]]
                    dw = [[128, 16], [2, 64], [1, 1]]
                    TTv(out=_ap(S[2], 0, 128, wo, dw),
                        in0=_ap(wb_t, 0, 128, ro, dr),
                        in1=_ap(wb_t, 0, 128, 129 + ro, dr), op=ADD)
                    CPs(out=_ap(S[2], 0, 128, 1 + wo, dw),
                        in_=_ap(wb_t, 0, 128, 1 + ro, dr))
                    CPv(out=_ap(T[2], 0, 128, 1 + wo, dw),
                        in_=_ap(wb_t, 0, 128, 128 + ro, dr))
                # levels b -> B
                for b, B in zip(SCALES[:-1], SCALES[1:]):
                    TTv(out=tmpap(b), in0=rd(S[b], b, 0, 0, 0, b), in1=rd(S[b], b, 1, 1, 0, b), op=ADD)
                    TTv(out=wr(S[B], B, 0), in0=tmpap(b), in1=rd(T[b], b, 0, 1, 0, b), op=ADD)
                    CPv(out=wr(S[B], B, 1), in_=rd(S[b], b, 0, 1, 0, b))
                    CPv(out=wr(T[B], B, 0), in_=rd(T[b], b, 1, 0, 0, b))
                    TTv(out=tmpap(b), in0=rd(T[b], b, 0, 0, 0, b), in1=rd(S[b], b, 1, 0, 0, b), op=ADD)
                    TTv(out=wr(T[B], B, 1), in0=tmpap(b), in1=rd(T[b], b, 1, 1, 0, b), op=ADD)

            # acc = a0*W can run as soon as wb lands; the G2 adds join later
            nc.vector.tensor_scalar_mul(dfull(acc_t, 1), dfull(wb_t, 1), a_bc[:, 0:1])

            # ---------- down-sweep (signed-lag Ghat pyramid) ----------

            # Ghat_64 init: G64[phat, 0, lam+64] = t2_64[lam mod 64]
            TTv(out=dfull(d_t, 64), in0=dfull(S[64], 64), in1=dfull(T[64], 64), op=ADD)
            nc.vector.tensor_scalar(out=dfull(t2_t, 64), in0=dfull(d_t, 64),
                                    scalar1=a_bc[:, 6:7], scalar2=1.0 / 64,
                                    op0=MULT, op1=MULT)
            CPv(out=_ap(G[64], 0, 128, 64, [[128, 2], [1, 64]]),
                in_=_ap(t2_t, 0, 128, 0, [[64, 2], [1, 64]]))
            CPs(out=_ap(G[64], 0, 128, 1, [[128, 2], [1, 63]]),
                in_=_ap(t2_t, 0, 128, 1, [[64, 2], [1, 63]]))

            for bi in range(len(SCALES) - 2, -1, -1):
                b, B = SCALES[bi], SCALES[bi + 1]
                idx = bi + 1
                TTv(out=dfull(d_t, b), in0=dfull(S[b], b), in1=dfull(T[b], b), op=ADD)
                nc.vector.tensor_scalar(out=dfull(t2_t, b), in0=dfull(d_t, b),
                                        scalar1=a_bc[:, idx:idx + 1], scalar2=1.0 / b,
                                        op0=MULT, op1=MULT)
                for eu in range(2):
                    for ev in range(2):
                        for sgn in range(2):
                            # sgn 0: lam in [0, b) (cnt b, k0 = 0)
                            # sgn 1: lam in [-(b-1), -1] (cnt b-1, k0 = 1)
                            cnt = b if sgn == 0 else b - 1
                            if cnt == 0:
                                continue
                            lam0 = 0 if sgn == 0 else -(b - 1)
                            k0 = lam0 % b  # t2 k index at lam0
                            t2ap = _ap(t2_t, 0, 128,
                                       128 * eu + b * ev + k0,
                                       [[256, 32 // b], [2 * b, 64 // b], [1, cnt]])
                            gbap = _ap(G[B], 0, 128,
                                       lam0 + b * (ev - eu) + B,
                                       [[256, 32 // b], [4 * b, 64 // b], [1, cnt]])
                            if b > 2:
                                oap = _ap(G[b], 0, 128,
                                          256 * eu + 2 * b * ev + b + lam0,
                                          [[512, 32 // b], [4 * b, 64 // b], [1, cnt]])
                            elif sgn == 0:
                                # E0[(2*r2+phat)*64 + 2*s2 + lam], lam in {0,1}
                                oap = _ap(E0_t, 0, 128, 128 * eu + 2 * ev,
                                          [[256, 16], [4, 32], [1, 2]])
                            else:
                                # E1 even lanes get the lam=-1 values
                                oap = _ap(E1_t, 0, 128, 128 * eu + 2 * ev,
                                          [[256, 16], [4, 32], [1, 1]])
                            TTv(out=oap, in0=t2ap, in1=gbap, op=ADD)
                if b == 2:
                    # E1 odd lanes = lam 0 values = E0 even lanes (off-DVE)
                    CPs(out=_ap(E1_t, 0, 128, 1, [[2, 2048]]),
                        in_=_ap(E0_t, 0, 128, 0, [[2, 2048]]))

            # ---------- final: acc += G2[r2, s2, (s%2 - r%2) + 2] ----------
            # split in free-halves (r2 < 16 vs >= 16) so the first half-store
            # can launch while the second half is still adding
            for hf in range(2):
                fo = hf * 4096    # acc free-half (r2 in [16*hf, 16*hf+16))
                eo = hf * 2048    # E-array half
                for er in range(2):
                    Ei = E0_t if er == 0 else E1_t
                    oap = _ap(acc_t, 0, 128, fo + 128 * er, [[256, 16], [1, 128]])
                    eap = _ap(Ei, 0, 128, eo, [[128, 16], [1, 128]])
                    TTv(out=oap, in0=oap, in1=eap, op=ADD)
                # alternate the two HWDGE queues so each group's half-pair
                # lands in parallel ahead of its gating transpose
                st_eng = nc.sync if hf == 0 else nc.scalar
                for g in range(4):
                    st_eng.dma_start(
                        out=bass.AP(tensor=w_flat, offset=g * 32 * 8192 + fo,
                                    ap=[[8192, 32], [1, 4096]]),
                        in_=acc_t.ap()[32 * g:32 * g + 32, fo:fo + 4096])

            # ---------- transposed reload: wT[ic] lhsT tiles ----------
            for ic in range(N_IC):
                nc.sync.dma_start(
                    out=wT[ic],
                    in_=bass.AP(tensor=w_flat, offset=ic * 65536, ap=[[128, 512], [1, 128]]),
                    transpose=True)

            # ---------- main matmul over token groups ----------
            for tg in range(n_tg):
                # one DMA loads all 16 ic-chunks for this token group
                xt = xt_pool.tile([128, N_IC * 512], bf16, name="xt")
                xt_inst = nc.scalar.dma_start(
                    out=xt[:],
                    in_=bass.AP(tensor=xT, offset=tg * 128 * (N_IC * 512),
                                ap=[[N_IC * 512, 128], [1, N_IC * 512]]))
                if tg < 2:
                    # keep the first prefetches off the SDMA until the
                    # build-gating weight load has landed
                    for wi in wsb_insts:
                        bass._add_dep_helper(xt_inst.ins, wi.ins, sync=True,
                                             reason="xt prefetch after wsb load")
                psums = [psum_pool.tile([128, 512], f32, name=f"ps{o}", tag="ps")
                         for o in range(N_OS)]
                for ic in range(N_IC):
                    rhs = xt[:, ic * 512:(ic + 1) * 512]
                    for o in range(N_OS):
                        nc.tensor.matmul(psums[o][:], wT[ic][:, o * 128:(o + 1) * 128], rhs,
                                         start=(ic == 0), stop=(ic == N_IC - 1))
                for o in range(N_OS):
                    ot = osb_pool.tile([128, 512], bf16, name="ot")
                    nc.scalar.activation(out=ot[:], in_=psums[o][:],
                                         func=mybir.ActivationFunctionType.Identity,
                                         bias=bias_sb[:, o:o + 1], scale=1.0)
                    nc.sync.dma_start(out=out.ap()[o * 128:(o + 1) * 128, tg * 512:(tg + 1) * 512],
                                      in_=ot[:])


        with (
            tc.tile_pool(name="xt", bufs=3) as xt_pool,
            tc.tile_pool(name="psum", bufs=8, space="PSUM") as psum_pool,
            tc.tile_pool(name="osb", bufs=8) as osb_pool,
        ):
            # reps>1 is a timing-measurement mode (serial full-body repeats)
            for _rep in range(reps):
                _one_pass(xt_pool, psum_pool, osb_pool)

    nc.compile()
    return nc


def make_ws_build(weight_f32):
    """Per-o-shard build-layout weight: [128, 8192] bf16,
    [pi = P*8+q, r*128 + phat*64 + s] = W[o*512 + q*64 + r, P*128 + phat*64 + s]."""
    import ml_dtypes
    outs = []
    for o in range(O_SHARDS):
        Wo = weight_f32[o * ROWS:(o + 1) * ROWS]            # [512, 2048]
        t = Wo.reshape(8, 64, 16, 2, 64).transpose(2, 0, 1, 3, 4)  # (P,q,r,phat,s)
        outs.append(np.ascontiguousarray(t.reshape(128, 8192)).astype(ml_dtypes.bfloat16))
    return outs


def make_in_maps(x, weight, bias, alphas, gumbels, tok=TOK):
    import ml_dtypes
    t_sh = TOK_TOTAL // tok
    n_tg = tok // 512
    x2 = np.asarray(x, np.float32).reshape(TOK_TOTAL, IN_F)
    xTh = np.ascontiguousarray(x2.T).astype(ml_dtypes.bfloat16)   # [2048, 16384]
    xslices = []
    for t in range(t_sh):
        sl = xTh[:, t * tok:(t + 1) * tok]                        # [2048, tok]
        # -> [tg, p, ic, t]: per-tg contiguous per-partition runs
        pre = sl.reshape(N_IC, 128, n_tg, 512).transpose(2, 1, 0, 3)
        xslices.append(np.ascontiguousarray(pre.reshape(n_tg * 128, N_IC * 512)))
    weight = np.asarray(weight, np.float32)
    bias = np.asarray(bias, np.float32)
    wslices = make_ws_build(weight)
    bslices = [np.ascontiguousarray(bias[o * ROWS:(o + 1) * ROWS]).reshape(1, ROWS)
               for o in range(O_SHARDS)]
    al = np.asarray(alphas, np.float32).reshape(1, 7)
    gu = np.asarray(gumbels, np.float32).reshape(1, 7)
    in_maps = []
    for c in range(N_CORES):
        t, o = divmod(c, O_SHARDS)
        in_maps.append({"xT": xslices[t % t_sh], "wsb": wslices[o], "bias_s": bslices[o],
                        "alphas": al, "gumbels": gu})
    return in_maps


def kernel(x, weight, bias, alphas, gumbels):
    if "nc" not in _CACHE:
        _CACHE["nc"] = _build_nc()
    nc = _CACHE["nc"]
    in_maps = make_in_maps(x, weight, bias, alphas, gumbels)
    res = run_bass_kernel_spmd(nc, in_maps, core_ids=list(range(N_CORES)))
    row_blocks = []
    for o in range(O_SHARDS):
        row_blocks.append(np.concatenate(
            [res.results[t * O_SHARDS + o]["out"] for t in range(T_SHARDS)], axis=1))
    full_t = np.concatenate(row_blocks, axis=0)              # [2048, 16384] bf16
    return np.ascontiguousarray(full_t.T).astype(np.float32).reshape(BATCH, TOKENS, OUT_F)

